# revision 1
# baseline (speedup 1.0000x reference)
"""Causal self-attention Bass kernel for 8 TRN2 NeuronCores.

Problem: B=4, T=2048, C=1024, H=16 heads, head_dim=64, fp32.
    q = x @ Wq.T ; k = x @ Wk.T ; v = x @ Wv.T          (per head)
    att = softmax(mask(q k^T / 8))
    y = att @ v ; out = y @ Wp.T

Sharding (8 cores): 4-way data parallel over batch x 2-way tensor
parallel over heads. Core c handles batch c//2 and heads 8*(c%2)..+8.
Wq/Wk/Wv column-parallel, Wp row-parallel; the partial outputs of the
two head-halves of each batch are summed on the host (the "all-reduce"
of row-parallel Wp).

Device dataflow (all transposed, so no on-chip transposes are needed):
    xT [C, T] (host-pretransposed) ->
    qT/kT = WqT.T-slices @ xT   [512, T]  (pairs of heads on partitions)
    v     = xT.T-tiles @ WvT    [T, 512]
    scoresT[k, q] = kT.T @ qT   (k on partitions -> softmax sum over k
                                 via a ones-column appended to v)
    expT = exp(0.125 * scoresT) (no max subtraction: scores ~ N(0, 0.4))
    yT[d, q] (+ row of sums) = v_aug.T @ expT, accumulated over k tiles
    out[t, c] = yT.T-tiles @ WpT, accumulated over local j

Projections and the output projection compute in float32r (TensorE
full rate, ~1.5e-4 rel err); the attention core (q/k/v/exp operands)
uses bf16, whose fast weight load keeps the PE's mixed-shape
instruction stream at full rate. End-to-end l2 rel err ~2.5e-3.
"""

from contextlib import ExitStack

import numpy as np

import concourse.bass as bass
import concourse.tile as tile
from concourse import bacc, mybir

F32 = mybir.dt.float32
F32R = mybir.dt.float32r
BF16 = mybir.dt.bfloat16

B, T, C, H, D = 4, 2048, 1024, 16, 64
NCORES = 8
JL = 512            # local j dims per core (8 heads * 64)
NPAIR = 4           # local head pairs
CI = C // 128       # 8 c-tiles
NT = T // 128       # 16 t/k tiles
NQC = T // 512      # 4 q chunks

_CACHED_NC = None


def build_nc():
    nc = bacc.Bacc(None)

    xT = nc.dram_tensor("xT", [C, T], F32R, kind="ExternalInput")
    wqT = nc.dram_tensor("wqT", [C, JL], F32R, kind="ExternalInput")
    wkT = nc.dram_tensor("wkT", [C, JL], F32R, kind="ExternalInput")
    wvT = nc.dram_tensor("wvT", [C, JL], F32R, kind="ExternalInput")
    wpT = nc.dram_tensor("wpT", [JL, C], F32R, kind="ExternalInput")
    out = nc.dram_tensor("out", [T, C], F32, kind="ExternalOutput")
    # bounce buffer for broadcasting softmax reciprocals across partitions
    rcd = nc.dram_tensor("rcd", [NPAIR, NQC, 2, 512], F32)

    xT_r = xT.rearrange("(ci p) t -> p ci t", p=128)
    wq_r = wqT.rearrange("(ci p) j -> p ci j", p=128)
    wk_r = wkT.rearrange("(ci p) j -> p ci j", p=128)
    wv_r = wvT.rearrange("(ci p) j -> p ci j", p=128)
    wp_r = wpT.rearrange("(ji p) c -> p ji c", p=128)

    with tile.TileContext(nc) as tc, ExitStack() as ctx:
        pm = ctx.enter_context(tc.tile_pool(name="pm", bufs=1))
        qkp = ctx.enter_context(tc.tile_pool(name="qkp", bufs=1))
        expp = ctx.enter_context(tc.tile_pool(name="expp", bufs=3))
        bcp = ctx.enter_context(tc.tile_pool(name="bcp", bufs=1))
        rcp = ctx.enter_context(tc.tile_pool(name="rcp", bufs=1))
        stp = ctx.enter_context(tc.tile_pool(name="stp", bufs=4))
        gp = ctx.enter_context(tc.tile_pool(name="gp", bufs=2, space="PSUM"))
        yp = ctx.enter_context(tc.tile_pool(name="yp", bufs=4, space="PSUM"))

        # v with a ones column prepended per head (so the softmax sums land
        # on psum partition 0, where reciprocal_approx_fast works), plus 64
        # pad columns so every per-head lhsT can be read as [128, 128] --
        # NumWeights==128 enables the fast weight load path.
        VW = D + 1
        v_sb = pm.tile([128, NT, 8 * VW + 64], BF16)
        v_view = v_sb[:, :, 0 : 8 * VW].rearrange("p n (h w) -> p n h w", w=VW)
        # (causal triangles are zeroed post-exp via gpsimd.affine_select)
        ones_col = pm.tile([128, NT, 8, 1], F32)
        nc.vector.memset(ones_col[:], 1.0)
        nc.vector.tensor_copy(v_view[:, :, :, 0:1], ones_col[:])
        nc.vector.memset(v_sb[:, :, 8 * VW : 8 * VW + 64], 0.0)

        qT_all = qkp.tile([128, NPAIR, T], BF16, tag="qT_all")
        kT_all = qkp.tile([128, NPAIR, T], BF16, tag="kT_all")

        # ---- phase 1: projections (stream xT by t-chunks) -----------------
        with (
            tc.tile_pool(name="ph1w", bufs=1) as wpool,
            tc.tile_pool(name="ph1x", bufs=2) as xpool,
        ):
            wq_sb = wpool.tile([128, CI, JL], F32R, tag="wq")
            wk_sb = wpool.tile([128, CI, JL], F32R, tag="wk")
            wv_sb = wpool.tile([128, CI, JL], F32R, tag="wv")
            # split weight/x DMAs per c-tile so the first matmuls can start
            # as soon as the first slices land; queue order matters (FIFO per
            # ring), so interleave chunk-0 x right after wq
            for ci in range(CI):
                nc.sync.dma_start(wq_sb[:, ci, :], wq_r[:, ci, :])
            xt0 = xpool.tile([128, CI, 512], F32R, tag="xt")
            for ci in range(CI):
                nc.sync.dma_start(xt0[:, ci, :], xT_r[:, ci, 0:512])
            for ci in range(CI):
                nc.sync.dma_start(wk_sb[:, ci, :], wk_r[:, ci, :])
            for ci in range(CI):
                nc.sync.dma_start(wv_sb[:, ci, :], wv_r[:, ci, :])

            for tch in range(NQC):
                ts_ = slice(tch * 512, tch * 512 + 512)
                if tch == 0:
                    xt = xt0
                else:
                    xt = xpool.tile([128, CI, 512], F32R, tag="xt")
                    for ci in range(CI):
                        nc.sync.dma_start(xt[:, ci, :], xT_r[:, ci, ts_])

                for w_sb, dst in ((wq_sb, qT_all), (wk_sb, kT_all)):
                    for pr in range(NPAIR):
                        acc = gp.tile([128, 2, 512], F32, tag="g")
                        for ci in range(CI):
                            nc.tensor.matmul(
                                acc[:, 0, :],
                                w_sb[:, ci, pr * 128 : pr * 128 + 128],
                                xt[:, ci, :],
                                start=(ci == 0),
                                stop=(ci == CI - 1),
                            )
                        nc.vector.tensor_copy(dst[:, pr, ts_], acc[:, 0, :])

                for tl in range(4):
                    ti = tch * 4 + tl
                    acc = gp.tile([128, 2, 512], F32, tag="g")
                    for ci in range(CI):
                        nc.tensor.matmul(
                            acc[:, 0, :],
                            xt[:, ci, tl * 128 : tl * 128 + 128],
                            wv_sb[:, ci, :],
                            start=(ci == 0),
                            stop=(ci == CI - 1),
                        )
                    nc.vector.tensor_copy(
                        v_view[:, ti, :, 1 : D + 1],
                        acc[:, 0, :].rearrange("p (h d) -> p h d", d=D),
                    )

        # ---- phase 2: attention + output projection ----------------------
        with (
            tc.tile_pool(name="ph2", bufs=1) as p2,
            tc.tile_pool(name="outp", bufs=3) as outp,
        ):
            wp_sb = p2.tile([128, NPAIR, C], F32R, tag="wp")
            nc.sync.dma_start(wp_sb[:], wp_r[:])
            yT_all = p2.tile([128, NPAIR, T], F32R, tag="yT")

            for pr in range(NPAIR):
                qlo = qT_all[0:64, pr, :]
                qhi = qT_all[64:128, pr, :]
                klo = kT_all[0:64, pr, :]
                khi = kT_all[64:128, pr, :]
                for qc in range(NQC):
                    nkt = 4 * qc + 4
                    qs = slice(qc * 512, qc * 512 + 512)
                    yA = yp.tile([128, 512], F32, tag="y")
                    yB = yp.tile([128, 512], F32, tag="y")

                    # software pipeline: issue scores/exp for kt before the PV
                    # matmuls of kt-1, so the PE never waits on ACT's exp.
                    # lhsT is [128, 128] (head's ones+v then pad/next-head
                    # cols); psum rows 65..127 are don't-care junk.
                    def emit_pv(kt, e, nkt=nkt):
                        dt = kt - 4 * qc
                        lo = dt * 128 if dt > 0 else 0
                        nc.tensor.matmul(
                            yA[:, lo:512],
                            v_sb[:, kt, 2 * pr * VW : 2 * pr * VW + 128],
                            e[:, 0, lo:512],
                            start=(kt == 0),
                            stop=(kt == nkt - 1),
                        )
                        nc.tensor.matmul(
                            yB[:, lo:512],
                            v_sb[:, kt, (2 * pr + 1) * VW : (2 * pr + 1) * VW + 128],
                            e[:, 1, lo:512],
                            start=(kt == 0),
                            stop=(kt == nkt - 1),
                        )

                    prev = None
                    for kt in range(nkt):
                        dt = kt - 4 * qc
                        ks = slice(kt * 128, kt * 128 + 128)
                        g = gp.tile([128, 2, 512], F32, tag="g")
                        nc.tensor.matmul(
                            g[:, 0, :], klo[:, ks], qlo[:, qs], start=True, stop=True
                        )
                        nc.tensor.matmul(
                            g[:, 1, :], khi[:, ks], qhi[:, qs], start=True, stop=True
                        )
                        e = expp.tile([128, 2, 512], BF16, tag="e")
                        xlo = dt * 128 if dt > 0 else 0
                        nc.scalar.activation(
                            e[:, :, xlo:512],
                            g[:, :, xlo:512],
                            mybir.ActivationFunctionType.Exp,
                            scale=0.125,
                        )
                        if dt >= 0:
                            # zero the causal triangle (k > q) of the diagonal
                            # block, on the otherwise-idle gpsimd engine
                            bs = slice(dt * 128, dt * 128 + 128)
                            for h in (0, 1):
                                nc.gpsimd.affine_select(
                                    out=e[:, h, bs],
                                    in_=e[:, h, bs],
                                    compare_op=mybir.AluOpType.is_ge,
                                    fill=0.0,
                                    base=0,
                                    pattern=[[1, 128]],
                                    channel_multiplier=-1,
                                )
                        if prev is not None:
                            emit_pv(*prev)
                        prev = (kt, e)
                    emit_pv(*prev)
                    # normalize: y / rowsum (sums live in row 0 = partition 0)
                    rc = rcp.tile([1, 2, 512], F32, tag="rc")
                    nc.vector.reciprocal_approx_fast(rc[0:1, 0, :], yA[0:1, :])
                    nc.vector.reciprocal_approx_fast(rc[0:1, 1, :], yB[0:1, :])
                    bc = bcp.tile([D + 1, 2, 512], F32, tag="bc")
                    for h in (0, 1):
                        nc.sync.dma_start(rcd[pr, qc, h : h + 1, :], rc[0:1, h, :])
                        s = rcd[pr, qc, h, :]
                        src = bass.AP(
                            tensor=s.tensor,
                            offset=s.offset,
                            ap=[[0, D + 1]] + list(s.ap),
                        )
                        nc.sync.dma_start(bc[0 : D + 1, h, :], src)
                    # y rows live on partitions 1..64; engines need 32-aligned
                    # partition bases, so multiply rows 0..64 (row 0 is the
                    # sums row scaled by its own reciprocal -- discarded) and
                    # repartition rows 1..64 into yT_all via DMA
                    stgA = stp.tile([D + 1, 512], F32R, tag="stg")
                    stgB = stp.tile([D + 1, 512], F32R, tag="stg")
                    nc.vector.tensor_mul(
                        stgA[0 : D + 1, :], yA[0 : D + 1, :], bc[0 : D + 1, 0, :]
                    )
                    nc.vector.tensor_mul(
                        stgB[0 : D + 1, :], yB[0 : D + 1, :], bc[0 : D + 1, 1, :]
                    )
                    nc.sync.dma_start(yT_all[0:64, pr, qs], stgA[1 : D + 1, :])
                    nc.sync.dma_start(yT_all[64:128, pr, qs], stgB[1 : D + 1, :])

            # output projection: out[t, c] = sum_j yT[j, t] * wpT[j, c]
            for ti in range(NT):
                tss = slice(ti * 128, ti * 128 + 128)
                for cc in range(2):
                    cs = slice(cc * 512, cc * 512 + 512)
                    acc = gp.tile([128, 2, 512], F32, tag="g")
                    for ji in range(NPAIR):
                        nc.tensor.matmul(
                            acc[:, 0, :],
                            yT_all[:, ji, tss],
                            wp_sb[:, ji, cs],
                            start=(ji == 0),
                            stop=(ji == NPAIR - 1),
                        )
                    o = outp.tile([128, 512], F32, tag="o")
                    nc.vector.tensor_copy(o[:], acc[:, 0, :])
                    nc.sync.dma_start(out[tss, cs], o[:])

    nc.finalize()
    return nc


def _get_nc():
    global _CACHED_NC
    if _CACHED_NC is None:
        _CACHED_NC = build_nc()
    return _CACHED_NC


def kernel(x, Wq, Wk, Wv, Wp):
    from concourse.bass_utils import run_bass_kernel_spmd

    x = np.asarray(x, dtype=np.float32)
    Wq = np.asarray(Wq, dtype=np.float32)
    Wk = np.asarray(Wk, dtype=np.float32)
    Wv = np.asarray(Wv, dtype=np.float32)
    Wp = np.asarray(Wp, dtype=np.float32)

    nc = _get_nc()

    xT = [np.ascontiguousarray(x[b].T) for b in range(B)]
    wqT, wkT, wvT, wpT = [], [], [], []
    for hh in range(2):
        js = slice(JL * hh, JL * hh + JL)
        wqT.append(np.ascontiguousarray(Wq[js, :].T))
        wkT.append(np.ascontiguousarray(Wk[js, :].T))
        wvT.append(np.ascontiguousarray(Wv[js, :].T))
        wpT.append(np.ascontiguousarray(Wp[:, js].T))

    in_maps = []
    for c in range(NCORES):
        b, hh = c // 2, c % 2
        in_maps.append(
            {
                "xT": xT[b],
                "wqT": wqT[hh],
                "wkT": wkT[hh],
                "wvT": wvT[hh],
                "wpT": wpT[hh],
            }
        )

    res = run_bass_kernel_spmd(nc, in_maps, core_ids=list(range(NCORES)))

    out = np.empty((B, T, C), dtype=np.float32)
    for b in range(B):
        out[b] = res.results[2 * b]["out"] + res.results[2 * b + 1]["out"]
    return out



# revision 5
# speedup vs baseline: 1.1964x; 1.1964x over previous
"""Causal self-attention Bass kernel for 8 TRN2 NeuronCores.

Problem: B=4, T=2048, C=1024, H=16 heads, head_dim=64, fp32.
    q = x @ Wq.T ; k = x @ Wk.T ; v = x @ Wv.T          (per head)
    att = softmax(mask(q k^T / 8))
    y = att @ v ; out = y @ Wp.T

Sharding (8 cores): 4-way data parallel over batch x 2-way tensor
parallel over heads. Core c handles batch c//2 and heads 8*(c%2)..+8.
Wq/Wk/Wv column-parallel, Wp row-parallel; the partial outputs of the
two head-halves of each batch are summed on the host (the "all-reduce"
of row-parallel Wp).

v2 design notes (from trace analysis of v1, 406us):
  - Attention inner loop was paced by ACT's EXP (~1250ns/kt vs ~1100ns
    PE).  Fix: alternate exp between ACT (even kt, true exp) and DVE
    (odd kt, Schraudolph bf16 bit-hack: e_bits = round(s*23.083 +
    16248.5) as int16, reinterpreted bf16; adds ~5e-3 rel err).
  - PV matmuls stalled ~300ns/kt waiting on just-finished exps.  Fix:
    PV lags exp by 2 kt iterations.
  - Scores in BF16 PSUM (1 bank per kt instead of 2) allows 3-deep
    score buffering + a 5-slot shared f32 psum ring (yA/yB + out-proj
    accumulators) within the 8-bank budget.
  - Output projection interleaved into the attention loop (qc-outer
    loop order) instead of a serial 35us tail.
  - Everything bf16 (weights, x, yT): fast weight loads (FWL) keep the
    PE's LDWEIGHTS off the critical path (fp32 LDW was 224ns > the
    213ns matmul streaming time), and input DMA traffic halves.
"""

from contextlib import ExitStack

import numpy as np

import concourse.bass as bass
import concourse.tile as tile
from concourse import bacc, mybir

F32 = mybir.dt.float32
BF16 = mybir.dt.bfloat16
I16 = mybir.dt.int16

B, T, C, H, D = 4, 2048, 1024, 16, 64
NCORES = 8
JL = 512            # local j dims per core (8 heads * 64)
NPAIR = 4           # local head pairs
CI = C // 128       # 8 c-tiles
NT = T // 128       # 16 t/k tiles
NQC = T // 512      # 4 q chunks
VW = D + 1

# Schraudolph bf16 bit-hack exp: bits = s*0.125*log2(e)*128 + (127*128 - 7.5)
EXP_A = 0.125 * 1.4426950408889634 * 128.0
EXP_B = 127.0 * 128.0 - 7.5

_CACHED_NC = None


def build_nc():
    nc = bacc.Bacc(None)

    xT = nc.dram_tensor("xT", [C, T], BF16, kind="ExternalInput")
    wqT = nc.dram_tensor("wqT", [C, JL], BF16, kind="ExternalInput")
    wkT = nc.dram_tensor("wkT", [C, JL], BF16, kind="ExternalInput")
    wvT = nc.dram_tensor("wvT", [C, JL], BF16, kind="ExternalInput")
    wpT = nc.dram_tensor("wpT", [JL, C], BF16, kind="ExternalInput")
    out = nc.dram_tensor("out", [T, C], F32, kind="ExternalOutput")
    # bounce buffer for broadcasting softmax reciprocals across partitions
    rcd = nc.dram_tensor("rcd", [NPAIR, NQC, 2, 512], F32)

    xT_r = xT.rearrange("(ci p) t -> p ci t", p=128)
    wq_r = wqT.rearrange("(ci p) j -> p ci j", p=128)
    wk_r = wkT.rearrange("(ci p) j -> p ci j", p=128)
    wv_r = wvT.rearrange("(ci p) j -> p ci j", p=128)
    wp_r = wpT.rearrange("(ji p) c -> p ji c", p=128)

    with tile.TileContext(nc) as tc, ExitStack() as ctx:
        pm = ctx.enter_context(tc.tile_pool(name="pm", bufs=1))
        qkp = ctx.enter_context(tc.tile_pool(name="qkp", bufs=1))
        expp = ctx.enter_context(tc.tile_pool(name="expp", bufs=4))
        bcp = ctx.enter_context(tc.tile_pool(name="bcp", bufs=2))
        rcp = ctx.enter_context(tc.tile_pool(name="rcp", bufs=2))
        sab = ctx.enter_context(tc.tile_pool(name="sab", bufs=2))
        stp = ctx.enter_context(tc.tile_pool(name="stp", bufs=2))
        outp = ctx.enter_context(tc.tile_pool(name="outp", bufs=3))
        # PSUM: scores (f32, 2 banks each) x3 + shared f32 ring x2 = 8 banks
        gp = ctx.enter_context(tc.tile_pool(name="gp", bufs=3, space="PSUM"))
        psf = ctx.enter_context(tc.tile_pool(name="psf", bufs=2, space="PSUM"))

        # v with a ones column prepended per head (so the softmax sums land
        # on psum partition 0), plus 64 pad columns so every per-head lhsT
        # can be read as [128, 128] -- NumWeights==128 enables FWL.
        v_sb = pm.tile([128, NT, 8 * VW + 64], BF16)
        v_view = v_sb[:, :, 0 : 8 * VW].rearrange("p n (h w) -> p n h w", w=VW)
        ones_col = pm.tile([128, NT, 8, 1], F32)
        nc.vector.memset(ones_col[:], 1.0)
        nc.vector.tensor_copy(v_view[:, :, :, 0:1], ones_col[:])
        nc.vector.memset(v_sb[:, :, 8 * VW : 8 * VW + 64], 0.0)

        qT_all = qkp.tile([128, NPAIR, T], BF16, tag="qT_all")
        kT_all = qkp.tile([128, NPAIR, T], BF16, tag="kT_all")
        yT_all = qkp.tile([128, NPAIR, T], BF16, tag="yT_all")
        wp_sb = qkp.tile([128, NPAIR, C], BF16, tag="wp")

        # ---- phase 1: projections (stream xT by t-chunks) -----------------
        with (
            tc.tile_pool(name="ph1w", bufs=1) as wpool,
            tc.tile_pool(name="ph1x", bufs=2) as xpool,
        ):
            wq_sb = wpool.tile([128, CI, JL], BF16, tag="wq")
            wk_sb = wpool.tile([128, CI, JL], BF16, tag="wk")
            wv_sb = wpool.tile([128, CI, JL], BF16, tag="wv")
            # split weight/x DMAs per c-tile so the first matmuls can start
            # as soon as the first slices land; queue order matters (FIFO per
            # ring), so interleave chunk-0 x right after wq
            for ci in range(CI):
                nc.sync.dma_start(wq_sb[:, ci, :], wq_r[:, ci, :])
            xt0 = xpool.tile([128, CI, 512], BF16, tag="xt")
            for ci in range(CI):
                nc.sync.dma_start(xt0[:, ci, :], xT_r[:, ci, 0:512])
            for ci in range(CI):
                nc.sync.dma_start(wk_sb[:, ci, :], wk_r[:, ci, :])
            for ci in range(CI):
                nc.sync.dma_start(wv_sb[:, ci, :], wv_r[:, ci, :])
            nc.sync.dma_start(wp_sb[:], wp_r[:])

            for tch in range(NQC):
                ts_ = slice(tch * 512, tch * 512 + 512)
                if tch == 0:
                    xt = xt0
                else:
                    xt = xpool.tile([128, CI, 512], BF16, tag="xt")
                    for ci in range(CI):
                        nc.sync.dma_start(xt[:, ci, :], xT_r[:, ci, ts_])

                for w_sb, dst, eng in (
                    (wq_sb, qT_all, "v"),
                    (wk_sb, kT_all, "a"),
                ):
                    for pr in range(NPAIR):
                        acc = psf.tile([128, 512], F32, tag="ps")
                        for ci in range(CI):
                            nc.tensor.matmul(
                                acc[:],
                                w_sb[:, ci, pr * 128 : pr * 128 + 128],
                                xt[:, ci, :],
                                start=(ci == 0),
                                stop=(ci == CI - 1),
                            )
                        if eng == "v":
                            nc.vector.tensor_copy(dst[:, pr, ts_], acc[:])
                        else:
                            nc.scalar.copy(dst[:, pr, ts_], acc[:])

                for tl in range(4):
                    ti = tch * 4 + tl
                    acc = psf.tile([128, 512], F32, tag="ps")
                    for ci in range(CI):
                        nc.tensor.matmul(
                            acc[:],
                            xt[:, ci, tl * 128 : tl * 128 + 128],
                            wv_sb[:, ci, :],
                            start=(ci == 0),
                            stop=(ci == CI - 1),
                        )
                    nc.vector.tensor_copy(
                        v_view[:, ti, :, 1 : D + 1],
                        acc[:].rearrange("p (h d) -> p h d", d=D),
                    )

        # ---- phase 2: attention with interleaved output projection -------
        outq = []          # pending out-proj (ti, cc) chains
        n_chain = [0]

        def emit_outproj(ti, cc):
            tss = slice(ti * 128, ti * 128 + 128)
            cs = slice(cc * 512, cc * 512 + 512)
            acc2 = gp.tile([128, 2, 512], F32, tag="g")
            acc = acc2[:, 0, :]
            for ji in range(NPAIR):
                nc.tensor.matmul(
                    acc,
                    yT_all[:, ji, tss],
                    wp_sb[:, ji, cs],
                    start=(ji == 0),
                    stop=(ji == NPAIR - 1),
                )
            o = outp.tile([128, 512], F32, tag="o")
            if n_chain[0] % 2 == 0:
                nc.vector.tensor_copy(o[:], acc)
            else:
                nc.scalar.copy(o[:], acc)
            n_chain[0] += 1
            nc.sync.dma_start(out[tss, cs], o[:])

        for qc in range(NQC):
            qs = slice(qc * 512, qc * 512 + 512)
            for pr in range(NPAIR):
                qlo = qT_all[0:64, pr, :]
                qhi = qT_all[64:128, pr, :]
                klo = kT_all[0:64, pr, :]
                khi = kT_all[64:128, pr, :]
                nkt = 4 * qc + 4
                yA = psf.tile([128, 512], F32, tag="ps")
                yB = psf.tile([128, 512], F32, tag="ps")

                def emit_pv(kt, e, nkt=nkt, yA=yA, yB=yB, pr=pr, qc=qc):
                    dt = kt - 4 * qc
                    lo = dt * 128 if dt > 0 else 0
                    nc.tensor.matmul(
                        yA[:, lo:512],
                        v_sb[:, kt, 2 * pr * VW : 2 * pr * VW + 128],
                        e[:, 0, lo:512],
                        start=(kt == 0),
                        stop=(kt == nkt - 1),
                    )
                    nc.tensor.matmul(
                        yB[:, lo:512],
                        v_sb[:, kt, (2 * pr + 1) * VW : (2 * pr + 1) * VW + 128],
                        e[:, 1, lo:512],
                        start=(kt == 0),
                        stop=(kt == nkt - 1),
                    )

                pending = []
                for kt in range(nkt):
                    dt = kt - 4 * qc
                    xlo = dt * 128 if dt > 0 else 0
                    ks = slice(kt * 128, kt * 128 + 128)
                    qw = slice(qc * 512 + xlo, qc * 512 + 512)
                    g = gp.tile([128, 2, 512], F32, tag="g")
                    nc.tensor.matmul(
                        g[:, 0, xlo:512], klo[:, ks], qlo[:, qw],
                        start=True, stop=True,
                    )
                    nc.tensor.matmul(
                        g[:, 1, xlo:512], khi[:, ks], qhi[:, qw],
                        start=True, stop=True,
                    )
                    e = expp.tile([128, 2, 512], BF16, tag="e")
                    if kt % 2 == 0:
                        # true exp on ACT
                        nc.scalar.activation(
                            e[:, :, xlo:512],
                            g[:, :, xlo:512],
                            mybir.ActivationFunctionType.Exp,
                            scale=0.125,
                        )
                    else:
                        # Schraudolph bit-hack exp on DVE
                        nc.vector.tensor_scalar(
                            e[:, :, xlo:512].bitcast(I16),
                            g[:, :, xlo:512],
                            EXP_A,
                            EXP_B,
                            mybir.AluOpType.mult,
                            mybir.AluOpType.add,
                        )
                    if dt >= 0:
                        # zero the causal triangle (k > q) of the diagonal
                        # block, on the otherwise-idle gpsimd engine
                        bs = slice(dt * 128, dt * 128 + 128)
                        for h in (0, 1):
                            nc.gpsimd.affine_select(
                                out=e[:, h, bs],
                                in_=e[:, h, bs],
                                compare_op=mybir.AluOpType.is_ge,
                                fill=0.0,
                                base=0,
                                pattern=[[1, 128]],
                                channel_multiplier=-1,
                            )
                    pending.append((kt, e))
                    if len(pending) > 2:
                        emit_pv(*pending.pop(0))
                    if outq and kt % 2 == 1:
                        emit_outproj(*outq.pop(0))
                for item in pending:
                    emit_pv(*item)

                # normalize: y / rowsum (sums live on psum partition 0).
                # ACT copies rows 0..64 to SBUF (releases the psum slots),
                # DVE computes reciprocals, a DRAM bounce broadcasts them
                # across partitions, gpsimd applies the scale.
                sAB = sab.tile([D + 1, 2, 512], F32, tag="s")
                nc.scalar.copy(sAB[:, 0, :], yA[0 : D + 1, :])
                nc.scalar.copy(sAB[:, 1, :], yB[0 : D + 1, :])
                rc = rcp.tile([1, 2, 512], F32, tag="rc")
                nc.vector.reciprocal_approx_fast(rc[0:1, 0, :], sAB[0:1, 0, :])
                nc.vector.reciprocal_approx_fast(rc[0:1, 1, :], sAB[0:1, 1, :])
                bc = bcp.tile([D + 1, 2, 512], F32, tag="bc")
                for h in (0, 1):
                    nc.sync.dma_start(rcd[pr, qc, h : h + 1, :], rc[0:1, h, :])
                    s = rcd[pr, qc, h, :]
                    src = bass.AP(
                        tensor=s.tensor,
                        offset=s.offset,
                        ap=[[0, D + 1]] + list(s.ap),
                    )
                    nc.sync.dma_start(bc[0 : D + 1, h, :], src)
                stg = stp.tile([D + 1, 2, 512], BF16, tag="stg")
                nc.gpsimd.tensor_mul(stg[:], sAB[:], bc[:])
                nc.sync.dma_start(yT_all[0:64, pr, qs], stg[1 : D + 1, 0, :])
                nc.sync.dma_start(yT_all[64:128, pr, qs], stg[1 : D + 1, 1, :])

            for ti in range(qc * 4, qc * 4 + 4):
                for cc in range(2):
                    outq.append((ti, cc))

        while outq:
            emit_outproj(*outq.pop(0))

    nc.finalize()
    return nc


def _get_nc():
    global _CACHED_NC
    if _CACHED_NC is None:
        _CACHED_NC = build_nc()
    return _CACHED_NC


def kernel(x, Wq, Wk, Wv, Wp):
    import ml_dtypes
    from concourse.bass_utils import run_bass_kernel_spmd

    BF = ml_dtypes.bfloat16
    x = np.asarray(x, dtype=np.float32)
    Wq = np.asarray(Wq, dtype=np.float32)
    Wk = np.asarray(Wk, dtype=np.float32)
    Wv = np.asarray(Wv, dtype=np.float32)
    Wp = np.asarray(Wp, dtype=np.float32)

    nc = _get_nc()

    xT = [np.ascontiguousarray(x[b].T).astype(BF) for b in range(B)]
    wqT, wkT, wvT, wpT = [], [], [], []
    for hh in range(2):
        js = slice(JL * hh, JL * hh + JL)
        wqT.append(np.ascontiguousarray(Wq[js, :].T).astype(BF))
        wkT.append(np.ascontiguousarray(Wk[js, :].T).astype(BF))
        wvT.append(np.ascontiguousarray(Wv[js, :].T).astype(BF))
        wpT.append(np.ascontiguousarray(Wp[:, js].T).astype(BF))

    in_maps = []
    for c in range(NCORES):
        b, hh = c // 2, c % 2
        in_maps.append(
            {
                "xT": xT[b],
                "wqT": wqT[hh],
                "wkT": wkT[hh],
                "wvT": wvT[hh],
                "wpT": wpT[hh],
            }
        )

    res = run_bass_kernel_spmd(nc, in_maps, core_ids=list(range(NCORES)))

    out = np.empty((B, T, C), dtype=np.float32)
    for b in range(B):
        out[b] = res.results[2 * b]["out"] + res.results[2 * b + 1]["out"]
    return out


# revision 6
# speedup vs baseline: 1.1986x; 1.0018x over previous
"""Causal self-attention Bass kernel for 8 TRN2 NeuronCores.

Problem: B=4, T=2048, C=1024, H=16 heads, head_dim=64, fp32.
    q = x @ Wq.T ; k = x @ Wk.T ; v = x @ Wv.T          (per head)
    att = softmax(mask(q k^T / 8))
    y = att @ v ; out = y @ Wp.T

Sharding (8 cores): 4-way data parallel over batch x 2-way tensor
parallel over heads. Core c handles batch c//2 and heads 8*(c%2)..+8.
Wq/Wk/Wv column-parallel, Wp row-parallel; the partial outputs of the
two head-halves of each batch are summed on the host (the "all-reduce"
of row-parallel Wp).

v2 design notes (from trace analysis of v1, 406us):
  - Attention inner loop was paced by ACT's EXP (~1250ns/kt vs ~1100ns
    PE).  Fix: alternate exp between ACT (even kt, true exp) and DVE
    (odd kt, Schraudolph bf16 bit-hack: e_bits = round(s*23.083 +
    16248.5) as int16, reinterpreted bf16; adds ~5e-3 rel err).
  - PV matmuls stalled ~300ns/kt waiting on just-finished exps.  Fix:
    PV lags exp by 2 kt iterations.
  - Scores in BF16 PSUM (1 bank per kt instead of 2) allows 3-deep
    score buffering + a 5-slot shared f32 psum ring (yA/yB + out-proj
    accumulators) within the 8-bank budget.
  - Output projection interleaved into the attention loop (qc-outer
    loop order) instead of a serial 35us tail.
  - Everything bf16 (weights, x, yT): fast weight loads (FWL) keep the
    PE's LDWEIGHTS off the critical path (fp32 LDW was 224ns > the
    213ns matmul streaming time), and input DMA traffic halves.
"""

from contextlib import ExitStack

import numpy as np

import concourse.bass as bass
import concourse.tile as tile
from concourse import bacc, mybir

F32 = mybir.dt.float32
BF16 = mybir.dt.bfloat16
I16 = mybir.dt.int16

B, T, C, H, D = 4, 2048, 1024, 16, 64
NCORES = 8
JL = 512            # local j dims per core (8 heads * 64)
NPAIR = 4           # local head pairs
CI = C // 128       # 8 c-tiles
NT = T // 128       # 16 t/k tiles
NQC = T // 512      # 4 q chunks
VW = D + 1

# Schraudolph bf16 bit-hack exp: bits = s*0.125*log2(e)*128 + (127*128 - 7.5)
EXP_A = 0.125 * 1.4426950408889634 * 128.0
EXP_B = 127.0 * 128.0 - 7.5

_CACHED_NC = None


def build_nc():
    nc = bacc.Bacc(None)

    xT = nc.dram_tensor("xT", [C, T], BF16, kind="ExternalInput")
    wqT = nc.dram_tensor("wqT", [C, JL], BF16, kind="ExternalInput")
    wkT = nc.dram_tensor("wkT", [C, JL], BF16, kind="ExternalInput")
    wvT = nc.dram_tensor("wvT", [C, JL], BF16, kind="ExternalInput")
    wpT = nc.dram_tensor("wpT", [JL, C], BF16, kind="ExternalInput")
    out = nc.dram_tensor("out", [T, C], F32, kind="ExternalOutput")
    # bounce buffer for broadcasting softmax reciprocals across partitions
    rcd = nc.dram_tensor("rcd", [NPAIR, NQC, 2, 512], F32)

    xT_r = xT.rearrange("(ci p) t -> p ci t", p=128)
    wq_r = wqT.rearrange("(ci p) j -> p ci j", p=128)
    wk_r = wkT.rearrange("(ci p) j -> p ci j", p=128)
    wv_r = wvT.rearrange("(ci p) j -> p ci j", p=128)
    wp_r = wpT.rearrange("(ji p) c -> p ji c", p=128)

    with tile.TileContext(nc) as tc, ExitStack() as ctx:
        pm = ctx.enter_context(tc.tile_pool(name="pm", bufs=1))
        qkp = ctx.enter_context(tc.tile_pool(name="qkp", bufs=1))
        expp = ctx.enter_context(tc.tile_pool(name="expp", bufs=4))
        bcp = ctx.enter_context(tc.tile_pool(name="bcp", bufs=2))
        rcp = ctx.enter_context(tc.tile_pool(name="rcp", bufs=2))
        sab = ctx.enter_context(tc.tile_pool(name="sab", bufs=2))
        stp = ctx.enter_context(tc.tile_pool(name="stp", bufs=2))
        outp = ctx.enter_context(tc.tile_pool(name="outp", bufs=3))
        # PSUM: scores (f32, 2 banks each) x3 + shared f32 ring x2 = 8 banks
        gp = ctx.enter_context(tc.tile_pool(name="gp", bufs=3, space="PSUM"))
        psf = ctx.enter_context(tc.tile_pool(name="psf", bufs=2, space="PSUM"))

        # v with a ones column prepended per head (so the softmax sums land
        # on psum partition 0), plus 64 pad columns so every per-head lhsT
        # can be read as [128, 128] -- NumWeights==128 enables FWL.
        v_sb = pm.tile([128, NT, 8 * VW + 64], BF16)
        v_view = v_sb[:, :, 0 : 8 * VW].rearrange("p n (h w) -> p n h w", w=VW)
        ones_col = pm.tile([128, NT, 8, 1], F32)
        nc.vector.memset(ones_col[:], 1.0)
        nc.vector.tensor_copy(v_view[:, :, :, 0:1], ones_col[:])
        nc.vector.memset(v_sb[:, :, 8 * VW : 8 * VW + 64], 0.0)

        qT_all = qkp.tile([128, NPAIR, T], BF16, tag="qT_all")
        kT_all = qkp.tile([128, NPAIR, T], BF16, tag="kT_all")
        yT_all = qkp.tile([128, NPAIR, T], BF16, tag="yT_all")
        wp_sb = qkp.tile([128, NPAIR, C], BF16, tag="wp")

        # ---- phase 1: projections (stream xT by t-chunks) -----------------
        with (
            tc.tile_pool(name="ph1w", bufs=1) as wpool,
            tc.tile_pool(name="ph1x", bufs=2) as xpool,
        ):
            wq_sb = wpool.tile([128, CI, JL], BF16, tag="wq")
            wk_sb = wpool.tile([128, CI, JL], BF16, tag="wk")
            wv_sb = wpool.tile([128, CI, JL], BF16, tag="wv")
            # split weight/x DMAs per c-tile so the first matmuls can start
            # as soon as the first slices land; queue order matters (FIFO per
            # ring), so interleave chunk-0 x right after wq
            for ci in range(CI):
                nc.sync.dma_start(wq_sb[:, ci, :], wq_r[:, ci, :])
            xt0 = xpool.tile([128, CI, 512], BF16, tag="xt")
            for ci in range(CI):
                nc.sync.dma_start(xt0[:, ci, :], xT_r[:, ci, 0:512])
            for ci in range(CI):
                nc.sync.dma_start(wk_sb[:, ci, :], wk_r[:, ci, :])
            for ci in range(CI):
                nc.sync.dma_start(wv_sb[:, ci, :], wv_r[:, ci, :])
            nc.sync.dma_start(wp_sb[:], wp_r[:])

            for tch in range(NQC):
                ts_ = slice(tch * 512, tch * 512 + 512)
                if tch == 0:
                    xt = xt0
                else:
                    xt = xpool.tile([128, CI, 512], BF16, tag="xt")
                    for ci in range(CI):
                        nc.sync.dma_start(xt[:, ci, :], xT_r[:, ci, ts_])

                for w_sb, dst, eng in (
                    (wq_sb, qT_all, "v"),
                    (wk_sb, kT_all, "a"),
                ):
                    for pr in range(NPAIR):
                        acc = psf.tile([128, 512], F32, tag="ps")
                        for ci in range(CI):
                            nc.tensor.matmul(
                                acc[:],
                                w_sb[:, ci, pr * 128 : pr * 128 + 128],
                                xt[:, ci, :],
                                start=(ci == 0),
                                stop=(ci == CI - 1),
                            )
                        if eng == "v":
                            nc.vector.tensor_copy(dst[:, pr, ts_], acc[:])
                        else:
                            nc.scalar.copy(dst[:, pr, ts_], acc[:])

                for tl in range(4):
                    ti = tch * 4 + tl
                    acc = psf.tile([128, 512], F32, tag="ps")
                    for ci in range(CI):
                        nc.tensor.matmul(
                            acc[:],
                            xt[:, ci, tl * 128 : tl * 128 + 128],
                            wv_sb[:, ci, :],
                            start=(ci == 0),
                            stop=(ci == CI - 1),
                        )
                    nc.vector.tensor_copy(
                        v_view[:, ti, :, 1 : D + 1],
                        acc[:].rearrange("p (h d) -> p h d", d=D),
                    )

        # ---- phase 2: attention with interleaved output projection -------
        outq = []          # pending out-proj (ti, cc) chains
        n_chain = [0]

        def emit_outproj(ti, cc):
            tss = slice(ti * 128, ti * 128 + 128)
            cs = slice(cc * 512, cc * 512 + 512)
            acc2 = gp.tile([128, 2, 512], F32, tag="g")
            acc = acc2[:, 0, :]
            for ji in range(NPAIR):
                nc.tensor.matmul(
                    acc,
                    yT_all[:, ji, tss],
                    wp_sb[:, ji, cs],
                    start=(ji == 0),
                    stop=(ji == NPAIR - 1),
                )
            o = outp.tile([128, 512], F32, tag="o")
            if n_chain[0] % 2 == 0:
                nc.vector.tensor_copy(o[:], acc)
            else:
                nc.scalar.copy(o[:], acc)
            n_chain[0] += 1
            nc.sync.dma_start(out[tss, cs], o[:])

        # deferred normalize tail: the gpsimd scale-multiply of (qc, pr) is
        # emitted after (qc, pr+1)'s kt loop so gpsimd's strict FIFO doesn't
        # delay the causal-mask affine_selects behind a 2.3us multiply.
        deferred = []

        def flush_deferred():
            while deferred:
                sAB, bc, stg, pr_, qs_ = deferred.pop(0)
                nc.gpsimd.tensor_mul(stg[:], sAB[:], bc[:])
                nc.sync.dma_start(yT_all[0:64, pr_, qs_], stg[1 : D + 1, 0, :])
                nc.sync.dma_start(
                    yT_all[64:128, pr_, qs_], stg[1 : D + 1, 1, :]
                )

        for qc in range(NQC):
            qs = slice(qc * 512, qc * 512 + 512)
            for pr in range(NPAIR):
                qlo = qT_all[0:64, pr, :]
                qhi = qT_all[64:128, pr, :]
                klo = kT_all[0:64, pr, :]
                khi = kT_all[64:128, pr, :]
                nkt = 4 * qc + 4
                yA = psf.tile([128, 512], F32, tag="ps")
                yB = psf.tile([128, 512], F32, tag="ps")

                def emit_pv(kt, e, nkt=nkt, yA=yA, yB=yB, pr=pr, qc=qc):
                    dt = kt - 4 * qc
                    lo = dt * 128 if dt > 0 else 0
                    nc.tensor.matmul(
                        yA[:, lo:512],
                        v_sb[:, kt, 2 * pr * VW : 2 * pr * VW + 128],
                        e[:, 0, lo:512],
                        start=(kt == 0),
                        stop=(kt == nkt - 1),
                    )
                    nc.tensor.matmul(
                        yB[:, lo:512],
                        v_sb[:, kt, (2 * pr + 1) * VW : (2 * pr + 1) * VW + 128],
                        e[:, 1, lo:512],
                        start=(kt == 0),
                        stop=(kt == nkt - 1),
                    )

                def emit_scores(kt, qc=qc, klo=klo, khi=khi, qlo=qlo, qhi=qhi):
                    dt = kt - 4 * qc
                    xlo = dt * 128 if dt > 0 else 0
                    ks = slice(kt * 128, kt * 128 + 128)
                    qw = slice(qc * 512 + xlo, qc * 512 + 512)
                    g = gp.tile([128, 2, 512], F32, tag="g")
                    nc.tensor.matmul(
                        g[:, 0, xlo:512], klo[:, ks], qlo[:, qw],
                        start=True, stop=True,
                    )
                    nc.tensor.matmul(
                        g[:, 1, xlo:512], khi[:, ks], qhi[:, qw],
                        start=True, stop=True,
                    )
                    e = expp.tile([128, 2, 512], BF16, tag="e")
                    if kt % 2 == 0:
                        # true exp on ACT
                        nc.scalar.activation(
                            e[:, :, xlo:512],
                            g[:, :, xlo:512],
                            mybir.ActivationFunctionType.Exp,
                            scale=0.125,
                        )
                    else:
                        # Schraudolph bit-hack exp on DVE
                        nc.vector.tensor_scalar(
                            e[:, :, xlo:512].bitcast(I16),
                            g[:, :, xlo:512],
                            EXP_A,
                            EXP_B,
                            mybir.AluOpType.mult,
                            mybir.AluOpType.add,
                        )
                    if dt >= 0:
                        # zero the causal triangle (k > q) of the diagonal
                        # block, on the otherwise-idle gpsimd engine
                        bs = slice(dt * 128, dt * 128 + 128)
                        for h in (0, 1):
                            nc.gpsimd.affine_select(
                                out=e[:, h, bs],
                                in_=e[:, h, bs],
                                compare_op=mybir.AluOpType.is_ge,
                                fill=0.0,
                                base=0,
                                pattern=[[1, 128]],
                                channel_multiplier=-1,
                            )
                    return e

                # kt loop in steps of 2: both scores pairs back-to-back on
                # the PE queue, then both lagged PV pairs, halving the
                # rhs-stream-switch refills (~105ns each).
                pending = []
                for kt0 in range(0, nkt, 2):
                    for kt in (kt0, kt0 + 1):
                        pending.append((kt, emit_scores(kt)))
                    # fill the pipeline-fill bubble at pr start with out-proj
                    if kt0 == 0:
                        for _ in range(2):
                            if outq:
                                emit_outproj(*outq.pop(0))
                    while len(pending) > 2:
                        emit_pv(*pending.pop(0))
                    if outq and kt0 % 4 == 2:
                        emit_outproj(*outq.pop(0))
                for item in pending:
                    emit_pv(*item)

                # normalize: y / rowsum (sums live on psum partition 0).
                # ACT copies rows 0..64 to SBUF (releases the psum slots),
                # DVE computes reciprocals, a DRAM bounce broadcasts them
                # across partitions, gpsimd applies the scale (deferred).
                sAB = sab.tile([D + 1, 2, 512], F32, tag="s")
                nc.scalar.copy(sAB[:, 0, :], yA[0 : D + 1, :])
                nc.scalar.copy(sAB[:, 1, :], yB[0 : D + 1, :])
                rc = rcp.tile([1, 2, 512], F32, tag="rc")
                nc.vector.reciprocal_approx_fast(rc[0:1, 0, :], sAB[0:1, 0, :])
                nc.vector.reciprocal_approx_fast(rc[0:1, 1, :], sAB[0:1, 1, :])
                bc = bcp.tile([D + 1, 2, 512], F32, tag="bc")
                for h in (0, 1):
                    nc.sync.dma_start(rcd[pr, qc, h : h + 1, :], rc[0:1, h, :])
                    s = rcd[pr, qc, h, :]
                    src = bass.AP(
                        tensor=s.tensor,
                        offset=s.offset,
                        ap=[[0, D + 1]] + list(s.ap),
                    )
                    nc.sync.dma_start(bc[0 : D + 1, h, :], src)
                stg = stp.tile([D + 1, 2, 512], BF16, tag="stg")
                deferred.append((sAB, bc, stg, pr, qs))
                if len(deferred) > 1:
                    sABd, bcd, stgd, pr_, qs_ = deferred.pop(0)
                    nc.gpsimd.tensor_mul(stgd[:], sABd[:], bcd[:])
                    nc.sync.dma_start(
                        yT_all[0:64, pr_, qs_], stgd[1 : D + 1, 0, :]
                    )
                    nc.sync.dma_start(
                        yT_all[64:128, pr_, qs_], stgd[1 : D + 1, 1, :]
                    )

            flush_deferred()
            for ti in range(qc * 4, qc * 4 + 4):
                for cc in range(2):
                    outq.append((ti, cc))

        while outq:
            emit_outproj(*outq.pop(0))

    nc.finalize()
    return nc


def _get_nc():
    global _CACHED_NC
    if _CACHED_NC is None:
        _CACHED_NC = build_nc()
    return _CACHED_NC


def kernel(x, Wq, Wk, Wv, Wp):
    import ml_dtypes
    from concourse.bass_utils import run_bass_kernel_spmd

    BF = ml_dtypes.bfloat16
    x = np.asarray(x, dtype=np.float32)
    Wq = np.asarray(Wq, dtype=np.float32)
    Wk = np.asarray(Wk, dtype=np.float32)
    Wv = np.asarray(Wv, dtype=np.float32)
    Wp = np.asarray(Wp, dtype=np.float32)

    nc = _get_nc()

    xT = [np.ascontiguousarray(x[b].T).astype(BF) for b in range(B)]
    wqT, wkT, wvT, wpT = [], [], [], []
    for hh in range(2):
        js = slice(JL * hh, JL * hh + JL)
        wqT.append(np.ascontiguousarray(Wq[js, :].T).astype(BF))
        wkT.append(np.ascontiguousarray(Wk[js, :].T).astype(BF))
        wvT.append(np.ascontiguousarray(Wv[js, :].T).astype(BF))
        wpT.append(np.ascontiguousarray(Wp[:, js].T).astype(BF))

    in_maps = []
    for c in range(NCORES):
        b, hh = c // 2, c % 2
        in_maps.append(
            {
                "xT": xT[b],
                "wqT": wqT[hh],
                "wkT": wkT[hh],
                "wvT": wvT[hh],
                "wpT": wpT[hh],
            }
        )

    res = run_bass_kernel_spmd(nc, in_maps, core_ids=list(range(NCORES)))

    out = np.empty((B, T, C), dtype=np.float32)
    for b in range(B):
        out[b] = res.results[2 * b]["out"] + res.results[2 * b + 1]["out"]
    return out
